# revision 25
# baseline (speedup 1.0000x reference)
"""Trainium2 Bass kernel for nn_Decompose_13477607375164.

The reference computation collapses to a per-image-plane 5x5 convolution:
    out = clip( sum_{i,j} w'[i,j] * clip(x,0,1)[.., r+i-2, c+j-2] + c', 0, 1 )
with reflect padding, where w'[i,j] = (wS_k . wE_k)/25 for k = i*5+j and
c' = (sum_k (wS_k . bE_k + bS_k)) / 25.

Strategy (pure data parallel over the 12 image planes, 8 cores):
  - Host: compute the 25 scalar taps + constant (tiny), clip x to [0,1] and
    convert to fp16 (the 2e-2 rel-err budget dwarfs fp16 rounding), reflect-
    pad, and pack each core's 1536 output rows into 13 row-groups of <= 123
    rows.  Each group ships as a ready-made [128, 1028] fp16 tile: partition
    0 is an all-ones row (carries the constant term), partitions 1.. hold
    the input rows of one or two contiguous row-blocks (groups may span
    half-plane boundaries; the banded stationary encodes the block layout).
  - Device: per group, the vertical taps are a banded-matrix matmul on the
    TensorEngine (stationary [128, mg] fp16 band), the horizontal taps are
    free-dim shifts of the moving operand; 5 shift-matmuls accumulate in a
    PSUM bank per 512-column chunk.  DVE clamps PSUM to [0,1] emitting fp16,
    and the fp16 result DMAs out.  A single fp16 pass (5 matmuls/chunk)
    replaces the baseline's fp32r+2xbf16 scheme (15 matmuls/chunk).
  - fp16 end-to-end traffic halves HBM bytes vs fp32; host up/down-converts.
"""

import numpy as np

import concourse.bacc as bacc
import concourse.mybir as mybir
from concourse.tile import TileContext
from concourse.bass_utils import run_bass_kernel_spmd

BS, C, H, W = 4, 3, 1024, 1024
SIZE = 5
PAD = 2
NCORES = 8
NRUN = 3             # half-planes per core
RUN_OUT = 512        # output rows per half-plane run
RUN_IN = RUN_OUT + 2 * PAD    # 516
INCOLS = W + 2 * PAD          # 1028
KDIM = 128
MG = 123             # max output rows per group
NCHUNK = 512
TOTROWS = NRUN * RUN_OUT      # 1536 output rows per core

F32 = mybir.dt.float32
F16 = mybir.dt.float16

_prog_cache = {}

# Number of on-device repetitions of the whole computation (used only for
# differential HW-time measurement from test.py; grading uses 1 = no loop).
REPEAT = 1
STAGGERED = True   # staggered For_i engine reset: cheaper loop back-edge
import os as _os
VARIANT = _os.environ.get("KVARIANT", "full")  # production; others are ablations


def _build_groups():
    """Pack 3 runs x 512 output rows into groups of <=123 rows / <=128 input
    partitions (1 ones-row + per-block (rows+4) halo).  Returns a list of
    groups; each group is (out_row0, [(run, pos, nrows), ...])."""
    groups = []
    cur = []
    row0 = 0
    nxt = 0
    for run in range(NRUN):
        pos = 0
        while pos < RUN_OUT:
            parts_left = 127 - sum(n + 4 for _, _, n in cur)
            cols_left = MG - sum(n for _, _, n in cur)
            n = min(RUN_OUT - pos, parts_left - 4, cols_left)
            if n <= 0:
                groups.append((row0, cur))
                row0 = nxt
                cur = []
                continue
            cur.append((run, pos, n))
            pos += n
            nxt += n
    groups.append((row0, cur))
    return groups


GROUPS = _build_groups()
NG = len(GROUPS)     # 13

# dedupe band stationaries: groups with identical block shapes share a band
_sigs = []
G2T = []
for _row0, _blocks in GROUPS:
    _sig = tuple(n for _, _, n in _blocks)
    if _sig not in _sigs:
        _sigs.append(_sig)
    G2T.append(_sigs.index(_sig))
NT = len(_sigs)      # 4 distinct band types

# chunking for the big-DMA layout: input chunks of groups, output chunks
IN_CHUNKS = [(0, 5), (5, 4), (9, 4)]
OUT_CHUNKS = [(0, 7), (7, 6)]


def _build_program_big(repeat=1, variant="big"):
    """Few big DMAs: partition-major flat layouts, chunked + double-buffered."""
    nc = bacc.Bacc(None, target_bir_lowering=False, debug=True)
    sw = 128 if variant == "big3" else MG   # stationary width (128 enables FWL)
    xgt = nc.dram_tensor("xgt", [KDIM, NG * INCOLS], F16, kind="ExternalInput")
    band = nc.dram_tensor("band", [KDIM, NG * SIZE * sw], F16, kind="ExternalInput")
    yt = nc.dram_tensor("yt", [MG, NG * W], F16, kind="ExternalOutput")

    do_mm = variant not in ("bigdmaonly",)
    do_clamp = do_mm

    from contextlib import ExitStack

    with TileContext(nc) as tc:
        with (
            tc.tile_pool(name="wconst", bufs=1) as cpool,
            tc.tile_pool(name="xp", bufs=2) as xpool,
            tc.tile_pool(name="op", bufs=2) as opool,
            tc.tile_pool(name="ps", bufs=6, space="PSUM") as pspool,
            ExitStack() as stack,
        ):
            bandt = cpool.tile([KDIM, NG * SIZE * sw], F16)
            nc.sync.dma_start(out=bandt[:, :], in_=band[:, :])

            srcbig = None
            if variant == "bigdmaonly":
                srcbig = cpool.tile([KDIM, 7 * W], F16)
                nc.vector.memset(srcbig[:, :], 0.25)

            if repeat > 1:
                stack.enter_context(
                    tc.For_i(
                        0, repeat, 1,
                        hint_engines=(
                            mybir.EngineType.PE,
                            mybir.EngineType.DVE,
                            mybir.EngineType.Activation,
                            mybir.EngineType.SP,
                        ),
                        staggered_reset=STAGGERED,
                    )
                )

            # issue input chunk loads up-front; double-buffered pools let
            # chunk c+1 land while chunk c computes
            xtiles = {}
            for ci, (g0, cnt) in enumerate(IN_CHUNKS):
                xt = xpool.tile([KDIM, 5 * INCOLS], F16, tag="xt")
                eng = nc.sync if ci % 2 == 0 else nc.scalar
                eng.dma_start(
                    out=xt[:, 0:cnt * INCOLS],
                    in_=xgt[:, g0 * INCOLS:(g0 + cnt) * INCOLS],
                )
                xtiles[ci] = xt

            if variant == "bigdmaonly":
                for ci, (g0, cnt) in enumerate(OUT_CHUNKS):
                    nc.gpsimd.dma_start(
                        out=yt[0:MG, g0 * W:(g0 + cnt) * W],
                        in_=srcbig[0:MG, 0:cnt * W],
                    )
            else:
                og_tiles = {}
                for ci, (g0, cnt) in enumerate(OUT_CHUNKS):
                    og_tiles[ci] = opool.tile([KDIM, 7 * W], F16, tag="og", name=f"og{ci}")

                def in_chunk_of(g):
                    for ci, (g0, cnt) in enumerate(IN_CHUNKS):
                        if g0 <= g < g0 + cnt:
                            return ci, g - g0
                    raise AssertionError

                def out_chunk_of(g):
                    for ci, (g0, cnt) in enumerate(OUT_CHUNKS):
                        if g0 <= g < g0 + cnt:
                            return ci, g - g0
                    raise AssertionError

                for g, (row0, blocks) in enumerate(GROUPS):
                    mg = sum(n for _, _, n in blocks)
                    ici, ioff = in_chunk_of(g)
                    oci, ooff = out_chunk_of(g)
                    xt = xtiles[ici]
                    og = og_tiles[oci]
                    if variant in ("big2", "big3"):
                        # j-outer over both 512-chunks: one Ldweights feeds
                        # two matmuls (the PE reloads per matmul otherwise)
                        mo = sw if variant == "big3" else mg
                        ps0 = pspool.tile([KDIM, NCHUNK], F32, tag="ps")
                        ps1 = pspool.tile([KDIM, NCHUNK], F32, tag="ps")
                        for j in range(SIZE):
                            for n0, ps in ((0, ps0), (NCHUNK, ps1)):
                                nc.tensor.matmul(
                                    ps[0:mo, :],
                                    bandt[:, (g * SIZE + j) * sw:
                                          (g * SIZE + j) * sw + mo],
                                    xt[:, ioff * INCOLS + n0 + j:
                                       ioff * INCOLS + n0 + j + NCHUNK],
                                    start=(j == 0), stop=(j == SIZE - 1),
                                )
                        for n0, ps in ((0, ps0), (NCHUNK, ps1)):
                            nc.vector.tensor_scalar(
                                og[0:mg, ooff * W + n0:ooff * W + n0 + NCHUNK],
                                ps[0:mg, :], 0.0, 1.0,
                                mybir.AluOpType.max, mybir.AluOpType.min,
                            )
                    else:
                        for n0 in (0, NCHUNK):
                            ps = pspool.tile([KDIM, NCHUNK], F32, tag="ps")
                            for j in range(SIZE):
                                nc.tensor.matmul(
                                    ps[0:mg, :],
                                    bandt[:, (g * SIZE + j) * MG:
                                          (g * SIZE + j) * MG + mg],
                                    xt[:, ioff * INCOLS + n0 + j:
                                       ioff * INCOLS + n0 + j + NCHUNK],
                                    start=(j == 0), stop=(j == SIZE - 1),
                                )
                            nc.vector.tensor_scalar(
                                og[0:mg, ooff * W + n0:ooff * W + n0 + NCHUNK],
                                ps[0:mg, :], 0.0, 1.0,
                                mybir.AluOpType.max, mybir.AluOpType.min,
                            )
                    # last group of an output chunk: ship the chunk
                    g0, cnt = OUT_CHUNKS[oci]
                    if g == g0 + cnt - 1:
                        nc.gpsimd.dma_start(
                            out=yt[0:MG, g0 * W:(g0 + cnt) * W],
                            in_=og[0:MG, 0:cnt * W],
                        )
    nc.compile()
    return nc


def _build_program(repeat=1, variant="full"):
    if variant in ("big", "big2", "big3", "bigdmaonly"):
        return _build_program_big(repeat, variant)
    nc = bacc.Bacc(None, target_bir_lowering=False, debug=True)
    xg = nc.dram_tensor("xg", [NG, KDIM, INCOLS], F16, kind="ExternalInput")
    band = nc.dram_tensor("band", [KDIM, NT * SIZE * MG], F16, kind="ExternalInput")
    y = nc.dram_tensor("y", [TOTROWS, W], F16, kind="ExternalOutput")

    preload = variant in ("nodmain", "peonly")
    do_mm = variant not in ("dmaonly",)
    do_clamp = variant not in ("dmaonly", "peonly", "noclamp")
    do_out = variant not in ("nodmaout", "peonly")

    from contextlib import ExitStack

    with TileContext(nc) as tc:
        with (
            tc.tile_pool(name="wconst", bufs=1) as cpool,
            tc.tile_pool(name="xp", bufs=8) as xpool,
            tc.tile_pool(name="op", bufs=4) as opool,
            tc.tile_pool(name="ps", bufs=8, space="PSUM") as pspool,
            ExitStack() as stack,
        ):
            bandt = cpool.tile([KDIM, NT * SIZE * MG], F16)
            for bt in range(NT):
                nc.sync.dma_start(
                    out=bandt[:, bt * SIZE * MG:(bt + 1) * SIZE * MG],
                    in_=band[:, bt * SIZE * MG:(bt + 1) * SIZE * MG],
                )

            pre_tiles = []
            if preload:
                for g in range(NG):
                    pt = cpool.tile([KDIM, INCOLS], F16, name=f"pre{g}")
                    eng = nc.sync if g % 2 == 0 else nc.scalar
                    eng.dma_start(out=pt[:, :], in_=xg[g, :, :])
                    pre_tiles.append(pt)

            if repeat > 1:
                stack.enter_context(
                    tc.For_i(
                        0, repeat, 1,
                        hint_engines=(
                            mybir.EngineType.PE,
                            mybir.EngineType.DVE,
                            mybir.EngineType.Activation,
                            mybir.EngineType.SP,
                        ),
                        staggered_reset=STAGGERED,
                    )
                )

            # issue all input loads up-front (8-deep pool): the scheduler
            # can then run loads as far ahead of compute as buffers allow
            xraw_tiles = {}
            if not preload:
                for g in range(NG):
                    xr = xpool.tile([KDIM, INCOLS], F16, tag="xraw",
                                    name=f"xraw{g}")
                    # alternate the two HWDGE rings; scalar first so the
                    # band DMAs on sync don't delay group 0
                    eng = nc.scalar if g % 2 == 0 else nc.sync
                    eng.dma_start(out=xr[:, :], in_=xg[g, :, :])
                    xraw_tiles[g] = xr

            for g, (row0, blocks) in enumerate(GROUPS):
                mg = sum(n for _, _, n in blocks)
                if preload:
                    xraw = pre_tiles[g]
                else:
                    xraw = xraw_tiles[g]

                otw = opool.tile([KDIM, W], F16, tag="otw")
                for n0 in (0, NCHUNK):
                    if do_mm:
                        bt = G2T[g]
                        ps = pspool.tile([KDIM, NCHUNK], F32, tag="ps")
                        for j in range(SIZE):
                            nc.tensor.matmul(
                                ps[0:mg, :],
                                bandt[:, (bt * SIZE + j) * MG:(bt * SIZE + j) * MG + mg],
                                xraw[:, n0 + j:n0 + j + NCHUNK],
                                start=(j == 0), stop=(j == SIZE - 1),
                            )
                    if do_clamp:
                        nc.vector.tensor_scalar(
                            otw[0:mg, n0:n0 + NCHUNK], ps[0:mg, :], 0.0, 1.0,
                            mybir.AluOpType.max, mybir.AluOpType.min,
                        )
                if do_out:
                    srct = otw if do_clamp else xraw
                    if g == NG - 1 and do_clamp:
                        nc.gpsimd.dma_start(
                            out=y[row0:row0 + mg, 0:NCHUNK],
                            in_=srct[0:mg, 0:NCHUNK])
                        nc.gpsimd.dma_start(
                            out=y[row0:row0 + mg, NCHUNK:W],
                            in_=srct[0:mg, NCHUNK:W])
                    else:
                        nc.gpsimd.dma_start(
                            out=y[row0:row0 + mg, :], in_=srct[0:mg, 0:W]
                        )
    nc.compile()
    return nc


def _build_weights(wE, bE, wS, bS, sw=MG, dedupe=False):
    # match the reference's fp32 arithmetic for the tap values
    a32 = np.einsum("kd,kd->k", wS, wE).astype(np.float32)
    c32 = (np.einsum("kd,kd->k", wS, bE).astype(np.float32)
           + bS.astype(np.float32)).astype(np.float32)
    wp = (a32 / np.float32(SIZE * SIZE)).astype(np.float32).reshape(SIZE, SIZE)
    cprime = np.float32(c32.sum(dtype=np.float32) / np.float32(SIZE * SIZE))

    w16 = wp.astype(np.float16)
    c16 = np.float16(cprime)

    nslots = NT if dedupe else NG
    band = np.zeros((KDIM, nslots, SIZE, sw), np.float16)
    done = set()
    for g, (_, blocks) in enumerate(GROUPS):
        slot = G2T[g] if dedupe else g
        if slot in done:
            continue
        done.add(slot)
        mg = sum(n for _, _, n in blocks)
        band[0, slot, 0, 0:mg] = c16
        off = 1
        col = 0
        for (_, _, n) in blocks:
            idx = np.arange(n)
            for i in range(SIZE):
                for j in range(SIZE):
                    band[off + i + idx, slot, j, col + idx] = w16[i, j]
            off += n + 4
            col += n
    return band.reshape(KDIM, nslots * SIZE * sw)


def kernel(x, wE, bE, wS, bS, _trace=False):
    x = np.asarray(x, dtype=np.float32)
    planes = np.clip(x.reshape(BS * C, H, W), 0.0, 1.0).astype(np.float16)
    xp = np.pad(planes, ((0, 0), (PAD, PAD), (PAD, PAD)), mode="reflect")

    bigv = VARIANT in ("big", "big2", "big3", "bigdmaonly")
    band = _build_weights(
        np.asarray(wE, np.float32), np.asarray(bE, np.float32),
        np.asarray(wS, np.float32), np.asarray(bS, np.float32),
        sw=128 if VARIANT == "big3" else MG, dedupe=not bigv,
    )

    big = VARIANT in ("big", "big2", "big3", "bigdmaonly")
    in_maps = []
    for core in range(NCORES):
        # padded input rows of this core's three half-plane runs
        runs = []
        for k in range(NRUN):
            h = core * NRUN + k          # half-plane index 0..23
            p, half = divmod(h, 2)
            runs.append(xp[p, half * RUN_OUT: half * RUN_OUT + RUN_IN, :])
        xg = np.zeros((NG, KDIM, INCOLS), np.float16)
        xg[:, 0, :] = np.float16(1.0)
        for g, (_, blocks) in enumerate(GROUPS):
            off = 1
            for (run, pos, n) in blocks:
                xg[g, off:off + n + 4, :] = runs[run][pos:pos + n + 4, :]
                off += n + 4
        if big:
            xgt = np.ascontiguousarray(xg.transpose(1, 0, 2)).reshape(
                KDIM, NG * INCOLS)
            in_maps.append({"xgt": xgt, "band": band})
        else:
            in_maps.append({"xg": xg, "band": band})

    key = ("prog", REPEAT, STAGGERED, VARIANT)
    if key not in _prog_cache:
        _prog_cache[key] = _build_program(REPEAT, VARIANT)
    nc = _prog_cache[key]

    res = run_bass_kernel_spmd(
        nc, in_maps, core_ids=list(range(NCORES)), trace=bool(_trace)
    )

    out = np.empty((BS * C, H, W), np.float32)
    for core in range(NCORES):
        if big:
            ytc = res.results[core]["yt"].reshape(MG, NG, W)
            yf = np.empty((TOTROWS, W), np.float32)
            for g, (row0, blocks) in enumerate(GROUPS):
                mg = sum(n for _, _, n in blocks)
                yf[row0:row0 + mg] = ytc[0:mg, g, :]
            yc = yf.reshape(NRUN, RUN_OUT, W)
        else:
            yc = res.results[core]["y"].astype(np.float32).reshape(
                NRUN, RUN_OUT, W)
        for k in range(NRUN):
            h = core * NRUN + k
            p, half = divmod(h, 2)
            out[p, half * RUN_OUT:(half + 1) * RUN_OUT, :] = yc[k]
    out = out.reshape(BS, C, H, W)

    if _trace:
        return out, res
    return out
